# revision 1
# baseline (speedup 1.0000x reference)
"""Trainium2 Bass kernel for nn_CoreAttention (S=2048, B=1, H=16, D=128).

Sharding: 16 heads across 8 NeuronCores (2 heads/core, tensor parallel).

Per head (big tensors stay feature-major so nothing large is transposed
on device; the host supplies Q^T/K^T/V^T per head):
    qT     = (Wqk^T Q^T) / NF            (bf16 PE, fp32 PSUM)
    kT     = Wqk^T K^T                   (bf16 PE)
    scoresT[k,q] = kT-block^T @ qT       (bf16 PE; causal: only q >= k)
    scoresT += causal mask on diag block (PE accumulate of -1e4 tile)
    expT   = exp(scoresT)                (ACT, PSUM->SBUF bf16)
    sums[q]= ones-matmuls over expT      (PE, N=1 column sums)
    v      = V^T-chunks^T @ Wv           (bf16 PE -> natural [s,e] layout)
    ctxT   = sum_j v_j^T @ expT_j        (bf16 PE, fp32 accum)
    ctx    = transpose(ctxT) * (1/sums)  (fp32 PE transpose + DVE scale)

exp() runs without max-subtraction: scores are ~N(0,1) (the reference
normalizes by sqrt(128)), so exp never overflows and matches the
reference's masked softmax to rounding error.
"""

import sys
from contextlib import ExitStack

import numpy as np

for _p in ("/opt/trn_rl_repo",):
    if _p not in sys.path:
        sys.path.insert(0, _p)

import ml_dtypes
import concourse.bass as bass
import concourse.tile as tile
from concourse import bacc, mybir
from concourse.bass_utils import run_bass_kernel_spmd

S, B, H, D = 2048, 1, 16, 128
HPC = 2  # heads per core
NCORES = 8
NB = S // 128  # 16 seq blocks of 128
NF = float(np.sqrt(2048.0 / 16.0))  # NORM_FACTOR
NEG = -10000.0
PAD = 384  # zero-pad columns in front of each expt_j buffer

F32 = mybir.dt.float32
BF16 = mybir.dt.bfloat16
AF = mybir.ActivationFunctionType


def build_program() -> bass.Bass:
    nc = bacc.Bacc(
        "TRN2", target_bir_lowering=False, debug=False, num_devices=NCORES
    )

    qt_d = nc.dram_tensor("qt", [HPC, D, S], F32, kind="ExternalInput")
    kt_d = nc.dram_tensor("kt", [HPC, D, S], F32, kind="ExternalInput")
    vt_d = nc.dram_tensor("vt", [HPC, D, S], F32, kind="ExternalInput")
    wqk_d = nc.dram_tensor("wqk", [HPC, D, D], F32, kind="ExternalInput")
    wv_d = nc.dram_tensor("wv", [HPC, D, D], F32, kind="ExternalInput")
    identf_d = nc.dram_tensor("identf", [D, D], F32, kind="ExternalInput")
    identb_d = nc.dram_tensor("identb", [D, D], BF16, kind="ExternalInput")
    maskb_d = nc.dram_tensor("maskb", [D, D], BF16, kind="ExternalInput")
    onesb_d = nc.dram_tensor("onesb", [D, 1], BF16, kind="ExternalInput")
    onesf_d = nc.dram_tensor("onesf", [1, 1], F32, kind="ExternalInput")
    out_d = nc.dram_tensor("out", [HPC, S, D], F32, kind="ExternalOutput")

    with tile.TileContext(nc) as tc, ExitStack() as ctx:
        cpool = ctx.enter_context(tc.tile_pool(name="const", bufs=1))
        sb = ctx.enter_context(tc.tile_pool(name="sb", bufs=1))
        ps = ctx.enter_context(tc.tile_pool(name="ps", bufs=1, space="PSUM"))

        identf = cpool.tile([D, D], F32)
        nc.sync.dma_start(identf[:], identf_d[:])
        identb = cpool.tile([D, D], BF16)
        nc.sync.dma_start(identb[:], identb_d[:])
        maskb = cpool.tile([D, D], BF16)
        nc.sync.dma_start(maskb[:], maskb_d[:])
        onesb = cpool.tile([D, 1], BF16)
        nc.sync.dma_start(onesb[:], onesb_d[:])
        onesf = cpool.tile([1, 1], F32)
        nc.sync.dma_start(onesf[:], onesf_d[:])

        # Warm the PE's view of identf's DMA queue so later fp32 transposes
        # (self-loading, max 1 sync wait) never need a second wait.
        warm_ps = ps.tile([D, D], F32, tag="otr", name="warm_ps")
        nc.tensor.transpose(warm_ps[:], identf[:], identf[:])

        for h in range(HPC):
            # ---- load raw inputs (weights first: tiny, unblock projs) -----
            wqk = sb.tile([D, D], F32, tag="wqk", bufs=2)
            nc.sync.dma_start(wqk[:], wqk_d[h])
            wv = sb.tile([D, D], F32, tag="wv", bufs=2)
            nc.sync.dma_start(wv[:], wv_d[h])
            wqkb = sb.tile([D, D], BF16, tag="wqkb", bufs=2)
            nc.vector.tensor_copy(wqkb[:], wqk[:])
            wvb = sb.tile([D, D], BF16, tag="wvb", bufs=2)
            nc.vector.tensor_copy(wvb[:], wv[:])

            # q/k/v loads + bf16 casts, pipelined at 1024-col granularity
            qtr = sb.tile([D, S], F32, tag="qtr", bufs=2)
            ktr = sb.tile([D, S], F32, tag="ktr", bufs=2)
            vtr = sb.tile([D, S], F32, tag="vtr", bufs=2)
            qtb = sb.tile([D, S], BF16, tag="qtb", bufs=1)
            ktb = sb.tile([D, S], BF16, tag="ktb", bufs=1)
            vtb = sb.tile([D, S], BF16, tag="vtb", bufs=1)
            for raw, dr, cast in ((qtr, qt_d, qtb), (ktr, kt_d, ktb), (vtr, vt_d, vtb)):
                for c in range(2):
                    sl = slice(c * 1024, (c + 1) * 1024)
                    nc.sync.dma_start(raw[:, sl], dr[h][:, sl])
                    nc.vector.tensor_copy(cast[:, sl], raw[:, sl])

            # ---- projections: qT = Wqk^T Q^T / NF,  kT = Wqk^T K^T --------
            qmt = sb.tile([D, S], BF16, tag="qmt", bufs=2)
            kmt = sb.tile([D, S], BF16, tag="kmt", bufs=2)
            for src, dst, scale in ((qtb, qmt, 1.0 / NF), (ktb, kmt, 1.0)):
                for c in range(2):
                    p = ps.tile(
                        [D, S // 2], F32, tag="big", bufs=2,
                        name=f"proj_ps_{h}_{dst.tensor.name}_{c}",
                    )
                    for c2 in range(2):
                        nc.tensor.matmul(
                            p[:, c2 * 512 : (c2 + 1) * 512],
                            wqkb[:],
                            src[:, c * 1024 + c2 * 512 : c * 1024 + (c2 + 1) * 512],
                        )
                    nc.scalar.activation(
                        dst[:, c * 1024 : (c + 1) * 1024], p[:], AF.Copy, scale=scale
                    )

            # ---- v chunks in natural [s,e] layout: v = V_raw @ Wv ---------
            vsb = sb.tile([D, NB * D], BF16, tag="vsb", bufs=2)
            for c in range(2):
                vp = ps.tile([D, S // 2], F32, tag="big", bufs=2, name=f"vp_ps_{h}_{c}")
                for j in range(8):
                    nc.tensor.matmul(
                        vp[:, j * 128 : (j + 1) * 128],
                        vtb[:, (c * 8 + j) * 128 : (c * 8 + j + 1) * 128],
                        wvb[:],
                    )
                nc.vector.tensor_copy(vsb[:, c * 1024 : (c + 1) * 1024], vp[:])

            # ---- pass 1: scoresT -> exp(bf16), left-padded with zeros -----
            # expt_j buffer holds PAD zero columns then the w real columns,
            # so later N=512 reads spanning "before the diagonal" see zeros.
            expts = []
            for j in range(NB):
                w = S - j * 128  # sq columns j*128 .. S
                expt = sb.tile(
                    [D, PAD + w], BF16, tag=f"expt{j}", bufs=2, name=f"expt_h{h}_{j}"
                )
                nc.gpsimd.memset(expt[:, 0:PAD], 0.0)
                nhalf = (w + 1023) // 1024
                for c in range(nhalf):
                    lo = c * 1024
                    cw = min(1024, w - lo)
                    sc_ps = ps.tile(
                        [D, cw], F32, tag="big", bufs=2, name=f"sc_ps_h{h}_{j}_{c}"
                    )
                    for c2 in range(0, cw, 512):
                        ce = min(c2 + 512, cw)
                        first = c == 0 and c2 == 0
                        nc.tensor.matmul(
                            sc_ps[:, c2:ce],
                            kmt[:, j * 128 : (j + 1) * 128],
                            qmt[:, j * 128 + lo + c2 : j * 128 + lo + ce],
                            start=True,
                            stop=not first,
                        )
                        if first:
                            # causal mask on diagonal block via PE accumulate
                            nc.tensor.matmul(
                                sc_ps[:, 0:128],
                                identb[:],
                                maskb[:],
                                start=False,
                                stop=True,
                            )
                    nc.scalar.activation(
                        expt[:, PAD + lo : PAD + lo + cw], sc_ps[:], AF.Exp
                    )
                expts.append(expt)

            # ---- softmax sums: ones-stationary N=512 row-sums -------------
            recip_ps = ps.tile([D, NB], F32, tag="recipps", name=f"recip_ps_{h}")
            for c in range(4):
                srow = ps.tile([1, 512], F32, tag="sumsrow", name=f"srow_{h}_{c}")
                njc = 4 * c + 4  # j = 0 .. 4c+3 contribute to this chunk
                for j in range(njc):
                    nc.tensor.matmul(
                        srow[:],
                        onesb[:],
                        expts[j][:, PAD + 512 * c - 128 * j : PAD + 512 * c - 128 * j + 512],
                        start=(j == 0),
                        stop=(j == njc - 1),
                    )
                srow_sb = sb.tile([1, 512], F32, tag="srow_sb", bufs=2)
                nc.vector.tensor_copy(srow_sb[:], srow[:])
                for s4 in range(4):
                    i = c * 4 + s4
                    # [1,128] row -> [128,1] column via K=1 matmul
                    nc.tensor.matmul(
                        recip_ps[:, i : i + 1],
                        srow_sb[0:1, s4 * 128 : (s4 + 1) * 128],
                        onesf[:],
                    )
            recip = sb.tile([D, NB], F32, tag="recip", bufs=2)
            nc.vector.reciprocal(recip[:], recip_ps[:])

            # ---- pass 2: PV accumulation, transpose, normalize, store -----
            for i4 in range(NB // 4):
                outt_ps = ps.tile([D, 512], F32, tag="outt", name=f"outt_{h}_{i4}")
                njc = 4 * i4 + 4
                for j in range(njc):
                    nc.tensor.matmul(
                        outt_ps[:],
                        vsb[:, j * 128 : (j + 1) * 128],
                        expts[j][:, PAD + 512 * i4 - 128 * j : PAD + 512 * i4 - 128 * j + 512],
                        start=(j == 0),
                        stop=(j == njc - 1),
                    )
                outt_sb = sb.tile([D, 512], F32, tag="outt_sb", bufs=2)
                nc.vector.tensor_copy(outt_sb[:], outt_ps[:])
                otr_ps = ps.tile([D, 512], F32, tag="otr", name=f"otr_{h}_{i4}")
                osb = sb.tile([D, 512], F32, tag="osb", bufs=2)
                for s4 in range(4):
                    i = i4 * 4 + s4
                    sl = slice(s4 * 128, (s4 + 1) * 128)
                    nc.tensor.transpose(otr_ps[:, sl], outt_sb[:, sl], identf[:])
                    nc.vector.tensor_scalar_mul(
                        osb[:, sl], otr_ps[:, sl], recip[:, i : i + 1]
                    )
                nc.sync.dma_start(
                    out_d[h, i4 * 512 : (i4 + 1) * 512, :].rearrange(
                        "(b s) e -> s b e", b=4
                    ),
                    osb[:].rearrange("p (b e) -> p b e", b=4),
                )

    nc.compile()
    return nc


_NC_CACHE = None


def _get_program():
    global _NC_CACHE
    if _NC_CACHE is None:
        _NC_CACHE = build_program()
    return _NC_CACHE


def make_in_maps(query_layer, key_layer, value_layer, svd_qk, svd_v):
    qt = np.ascontiguousarray(query_layer[:, 0].transpose(1, 2, 0))
    kt = np.ascontiguousarray(key_layer[:, 0].transpose(1, 2, 0))
    vt = np.ascontiguousarray(value_layer[:, 0].transpose(1, 2, 0))
    svd_qk = np.ascontiguousarray(svd_qk, dtype=np.float32)
    svd_v = np.ascontiguousarray(svd_v, dtype=np.float32)

    identf = np.eye(D, dtype=np.float32)
    identb = np.eye(D, dtype=ml_dtypes.bfloat16)
    r = np.arange(D)
    maskb = np.where(r[:, None] > r[None, :], NEG, 0.0).astype(ml_dtypes.bfloat16)
    onesb = np.ones((D, 1), dtype=ml_dtypes.bfloat16)

    in_maps = []
    for c in range(NCORES):
        hs = slice(c * HPC, (c + 1) * HPC)
        in_maps.append(
            {
                "qt": qt[hs],
                "kt": kt[hs],
                "vt": vt[hs],
                "wqk": svd_qk[hs],
                "wv": svd_v[hs],
                "identf": identf,
                "identb": identb,
                "maskb": maskb,
                "onesb": onesb,
                "onesf": np.ones((1, 1), dtype=np.float32),
            }
        )
    return in_maps


def assemble_output(results):
    out = np.empty((S, B, H * D), dtype=np.float32)
    for c in range(NCORES):
        o = results[c]["out"]  # [HPC, S, D]
        for hl in range(HPC):
            h = c * HPC + hl
            out[:, 0, h * D : (h + 1) * D] = o[hl]
    return out


def kernel(query_layer, key_layer, value_layer, attention_mask, svd_qk, svd_v):
    nc = _get_program()
    in_maps = make_in_maps(query_layer, key_layer, value_layer, svd_qk, svd_v)
    res = run_bass_kernel_spmd(nc, in_maps, list(range(NCORES))).results
    return assemble_output(res)



# revision 4
# speedup vs baseline: 1.4080x; 1.4080x over previous
"""Trainium2 Bass kernel for nn_CoreAttention (S=2048, B=1, H=16, D=128).

Sharding: 16 heads across 8 NeuronCores (2 heads/core, tensor parallel).

v2 design (vs baseline): natural-output PV with a fused softmax-sums
column, eliminating the separate ones-matmul sums pass, the fp32 PE
transposes, and the GpSimd PAD memsets.

Per head (feature-major host layout: Q^T/K^T/V^T [D, S]):
    qk_proj: psum = Wqk^T @ [Q^T | K^T]      (fp32r PE, raw fp32 inputs,
                                              no input casts; DVE casts
                                              psum -> bf16 qkmt)
    v:       v_nat[s,e] = V^T-chunk^T @ Wv   (bf16 PE -> [s, e] rows),
             stored as vaug[k, j, 0:129] with col 128 = 1.0
    scoresT[k,q] = kmt_j^T @ qmt chunks      (bf16 PE; causal stream,
                                              diag-mask added via PE
                                              accumulate of -1e4 tile)
    expT   = exp(scoresT / NF)               (ACT, psum -> bf16 SBUF,
                                              scale folded into exp)
    PV band i: ctx[q, 0:129] = sum_{j<=i} expT_block(i,j)^T @ vaug_j
             col 128 = softmax sums (ones column of vaug)
    out    = ctx[:, 0:128] * (1/ctx[:, 128]) (DVE recip + scalar mul)

exp() runs without max-subtraction: scores/NF are ~N(0,1), so exp never
overflows and matches the reference's masked softmax to rounding error.
"""

import sys
from contextlib import ExitStack

import numpy as np

for _p in ("/opt/trn_rl_repo",):
    if _p not in sys.path:
        sys.path.insert(0, _p)

import ml_dtypes
import concourse.bass as bass
import concourse.tile as tile
from concourse import bacc, mybir
from concourse.bass_utils import run_bass_kernel_spmd

S, B, H, D = 2048, 1, 16, 128
HPC = 2  # heads per core
NCORES = 8
NB = S // 128  # 16 seq blocks of 128
NF = float(np.sqrt(2048.0 / 16.0))  # NORM_FACTOR
NEG = -10000.0
TILE = 1536  # psum scores tile columns (3 banks)

F32 = mybir.dt.float32
F32R = mybir.dt.float32r
BF16 = mybir.dt.bfloat16
AF = mybir.ActivationFunctionType

# causal column stream: pass j emits scoresT columns for q in [128j, S)
OFFS = [0]
for j in range(NB):
    OFFS.append(OFFS[-1] + (S - 128 * j))
W = OFFS[NB]  # 17408 total causal columns per head


def build_program() -> bass.Bass:
    nc = bacc.Bacc(
        "TRN2", target_bir_lowering=False, debug=False, num_devices=NCORES
    )

    qt_d = nc.dram_tensor("qt", [HPC, D, S], F32R, kind="ExternalInput")
    kt_d = nc.dram_tensor("kt", [HPC, D, S], F32R, kind="ExternalInput")
    vt_d = nc.dram_tensor("vt", [HPC, D, S], F32, kind="ExternalInput")
    wqk_d = nc.dram_tensor("wqk", [HPC, D, D], F32R, kind="ExternalInput")
    wv_d = nc.dram_tensor("wv", [HPC, D, D], F32, kind="ExternalInput")
    identb_d = nc.dram_tensor("identb", [D, D], BF16, kind="ExternalInput")
    maskb_d = nc.dram_tensor("maskb", [D, D], BF16, kind="ExternalInput")
    out_d = nc.dram_tensor("out", [HPC, S, D], F32, kind="ExternalOutput")

    ntiles = (W + TILE - 1) // TILE
    tiles = [(t0, min(TILE, W - t0)) for t0 in range(0, W, TILE)]
    bands_by_tile = [[] for _ in range(ntiles)]
    for i in range(NB):
        bands_by_tile[OFFS[i] // TILE].append(i)

    with tile.TileContext(nc) as tc, ExitStack() as ctx:
        cpool = ctx.enter_context(tc.tile_pool(name="const", bufs=1))
        sb = ctx.enter_context(tc.tile_pool(name="sb", bufs=1))
        ps = ctx.enter_context(tc.tile_pool(name="ps", bufs=1, space="PSUM"))

        identb = cpool.tile([D, D], BF16)
        nc.sync.dma_start(identb[:], identb_d[:])
        maskb = cpool.tile([D, D], BF16)
        nc.sync.dma_start(maskb[:], maskb_d[:])

        # ---- all input DMAs up front (both heads) ---------------------
        wqk_t, wv_t, qkraw, vtr = {}, {}, {}, {}
        for h in range(HPC):
            wqk_t[h] = sb.tile([D, D], F32R, tag="wqk", bufs=2, name=f"wqk_{h}")
            nc.sync.dma_start(wqk_t[h][:], wqk_d[h])
            wv_t[h] = sb.tile([D, D], F32, tag="wv", bufs=2, name=f"wv_{h}")
            nc.sync.dma_start(wv_t[h][:], wv_d[h])
            qkraw[h] = sb.tile([D, 2 * S], F32R, tag="qkraw", bufs=2,
                               name=f"qkraw_{h}")
            for c in range(2):
                sl = slice(c * 1024, (c + 1) * 1024)
                nc.sync.dma_start(qkraw[h][:, sl], qt_d[h][:, sl])
            for c in range(2):
                sl = slice(c * 1024, (c + 1) * 1024)
                nc.sync.dma_start(
                    qkraw[h][:, S + c * 1024 : S + (c + 1) * 1024], kt_d[h][:, sl]
                )
            vtr[h] = sb.tile([D, S], F32, tag="vtr", bufs=2, name=f"vtr_{h}")
            for c in range(2):
                sl = slice(c * 1024, (c + 1) * 1024)
                nc.sync.dma_start(vtr[h][:, sl], vt_d[h][:, sl])

        for h in range(HPC):
            # ---- bf16 casts for the v path (gpsimd, SBUF->SBUF) -------
            wvb = sb.tile([D, D], BF16, tag="wvb", bufs=2, name=f"wvb_{h}")
            nc.gpsimd.tensor_copy(wvb[:], wv_t[h][:])
            vtb = sb.tile([D, S], BF16, tag="vtb", bufs=2, name=f"vtb_{h}")
            for c in range(2):
                sl = slice(c * 1024, (c + 1) * 1024)
                nc.gpsimd.tensor_copy(vtb[:, sl], vtr[h][:, sl])

            # ---- q/k projections: fp32r matmuls on raw inputs ---------
            qkmt = sb.tile([D, 2 * S], BF16, tag="qkmt", bufs=2,
                           name=f"qkmt_{h}")
            for t0, w in ((0, TILE), (TILE, TILE), (2 * TILE, 2 * S - 2 * TILE)):
                bigt = ps.tile([D, TILE], F32, tag="big", bufs=2,
                               name=f"projps_{h}_{t0}")
                for c in range(0, w, 256):
                    nc.tensor.matmul(
                        bigt[:, c : c + 256],
                        wqk_t[h][:],
                        qkraw[h][:, t0 + c : t0 + c + 256],
                        start=True,
                        stop=True,
                    )
                nc.vector.tensor_copy(qkmt[:, t0 : t0 + w], bigt[:, 0:w])

            # ---- v chunks in natural [s,e] layout + ones column -------
            vaug = sb.tile([D, NB, 132], BF16, tag="vaug", bufs=2,
                           name=f"vaug_{h}")
            nc.gpsimd.memset(vaug[:, :, 128:129], 1.0)
            for t0, w in ((0, TILE), (TILE, S - TILE)):
                bigt = ps.tile([D, TILE], F32, tag="big", bufs=2,
                               name=f"vps_{h}_{t0}")
                for c in range(0, w, 128):
                    nc.tensor.matmul(
                        bigt[:, c : c + 128],
                        vtb[:, t0 + c : t0 + c + 128],
                        wvb[:],
                        start=True,
                        stop=True,
                    )
                nj = w // 128
                nc.vector.tensor_copy(
                    vaug[:, t0 // 128 : t0 // 128 + nj, 0:128],
                    bigt[:, 0:w].rearrange("p (j c) -> p j c", c=128),
                )

            # ---- causal scores stream -> exp, with interleaved PV -----
            exp_all = sb.tile([D, W], BF16, tag="expall", bufs=2,
                              name=f"expall_{h}")

            def emit_pv(band_list):
                for i in band_list:
                    ctxp = ps.tile([D, 132], F32, tag="ctx", bufs=2,
                                   name=f"ctx_{h}_{i}")
                    for j in range(i + 1):
                        p = OFFS[j] + 128 * (i - j)
                        nc.tensor.matmul(
                            ctxp[:, 0:129],
                            exp_all[:, p : p + 128],
                            vaug[:, j, 0:129],
                            start=(j == 0),
                            stop=(j == i),
                        )
                    recip = sb.tile([D, 1], F32, tag="recip", bufs=2,
                                    name=f"recip_{h}_{i}")
                    nc.vector.reciprocal(recip[:], ctxp[:, 128:129])
                    if i % 2 == 0:
                        osb_box[0] = sb.tile([D, 2, 128], F32, tag="osb",
                                             bufs=2, name=f"osb_{h}_{i}")
                    osb = osb_box[0]
                    nc.vector.tensor_scalar_mul(
                        osb[:, i % 2, :], ctxp[:, 0:128], recip[:]
                    )
                    if i % 2 == 1:
                        nc.sync.dma_start(
                            out_d[h, (i - 1) * 128 : (i + 1) * 128, :].rearrange(
                                "(b s) e -> s b e", b=2
                            ),
                            osb[:],
                        )

            osb_box = [None]
            for t, (t0, w) in enumerate(tiles):
                bigt = ps.tile([D, TILE], F32, tag="big", bufs=2,
                               name=f"scps_{h}_{t}")
                pos = t0
                while pos < t0 + w:
                    # pass containing pos
                    j = 0
                    while OFFS[j + 1] <= pos:
                        j += 1
                    bank_end = t0 + ((pos - t0) // 512 + 1) * 512
                    end = min(OFFS[j + 1], t0 + w, bank_end)
                    qcol = 128 * j + (pos - OFFS[j])
                    is_start = pos == OFFS[j]
                    nc.tensor.matmul(
                        bigt[:, pos - t0 : end - t0],
                        qkmt[:, S + 128 * j : S + 128 * (j + 1)],
                        qkmt[:, qcol : qcol + (end - pos)],
                        start=True,
                        stop=not is_start,
                    )
                    if is_start:
                        # causal mask on diagonal block via PE accumulate
                        nc.tensor.matmul(
                            bigt[:, pos - t0 : pos - t0 + 128],
                            identb[:],
                            maskb[:],
                            start=False,
                            stop=True,
                        )
                    pos = end
                nc.scalar.activation(
                    exp_all[:, t0 : t0 + w], bigt[:, 0:w], AF.Exp,
                    scale=1.0 / NF,
                )
                if t >= 1:
                    emit_pv(bands_by_tile[t - 1])
            emit_pv(bands_by_tile[ntiles - 1])

    nc.compile()
    return nc


_NC_CACHE = None


def _get_program():
    global _NC_CACHE
    if _NC_CACHE is None:
        _NC_CACHE = build_program()
    return _NC_CACHE


def make_in_maps(query_layer, key_layer, value_layer, svd_qk, svd_v):
    qt = np.ascontiguousarray(query_layer[:, 0].transpose(1, 2, 0))
    kt = np.ascontiguousarray(key_layer[:, 0].transpose(1, 2, 0))
    vt = np.ascontiguousarray(value_layer[:, 0].transpose(1, 2, 0))
    svd_qk = np.ascontiguousarray(svd_qk, dtype=np.float32)
    svd_v = np.ascontiguousarray(svd_v, dtype=np.float32)

    identb = np.eye(D, dtype=ml_dtypes.bfloat16)
    r = np.arange(D)
    maskb = np.where(r[:, None] > r[None, :], NEG, 0.0).astype(ml_dtypes.bfloat16)

    in_maps = []
    for c in range(NCORES):
        hs = slice(c * HPC, (c + 1) * HPC)
        in_maps.append(
            {
                "qt": qt[hs],
                "kt": kt[hs],
                "vt": vt[hs],
                "wqk": svd_qk[hs],
                "wv": svd_v[hs],
                "identb": identb,
                "maskb": maskb,
            }
        )
    return in_maps


def assemble_output(results):
    out = np.empty((S, B, H * D), dtype=np.float32)
    for c in range(NCORES):
        o = results[c]["out"]  # [HPC, S, D]
        for hl in range(HPC):
            h = c * HPC + hl
            out[:, 0, h * D : (h + 1) * D] = o[hl]
    return out


def kernel(query_layer, key_layer, value_layer, attention_mask, svd_qk, svd_v):
    nc = _get_program()
    in_maps = make_in_maps(query_layer, key_layer, value_layer, svd_qk, svd_v)
    res = run_bass_kernel_spmd(nc, in_maps, list(range(NCORES))).results
    return assemble_output(res)


# revision 5
# speedup vs baseline: 1.4622x; 1.0385x over previous
"""Trainium2 Bass kernel for nn_CoreAttention (S=2048, B=1, H=16, D=128).

Sharding: 16 heads across 8 NeuronCores (2 heads/core, tensor parallel).

v3 design: natural-output PV with a fused softmax-sums column (129th
moving column of ones), causal scores stream with exp on ACT, and
aggressive cross-head software pipelining:
  - input DMAs chunked small-first and priority-ordered (one dma_start
    lands on ONE dma engine at ~22GB/s, so lead chunks are 128 cols)
  - head-1 q/k projections interleaved into head-0's scores stream
  - psum->sbuf drains split between DVE and ACT to avoid serializing
    against the projection matmuls
  - per-head output staging tile so out-DMA issue order never
    backpressures the band epilogues
"""

import sys
from contextlib import ExitStack

import numpy as np

for _p in ("/opt/trn_rl_repo",):
    if _p not in sys.path:
        sys.path.insert(0, _p)

import ml_dtypes
import concourse.bass as bass
import concourse.tile as tile
from concourse import bacc, mybir
from concourse.bass_utils import run_bass_kernel_spmd

S, B, H, D = 2048, 1, 16, 128
HPC = 2  # heads per core
NCORES = 8
NB = S // 128  # 16 seq blocks of 128
NF = float(np.sqrt(2048.0 / 16.0))  # NORM_FACTOR
NEG = -10000.0
TILE = 1536  # psum scores tile columns (3 banks)

F32 = mybir.dt.float32
F32R = mybir.dt.float32r
BF16 = mybir.dt.bfloat16
AF = mybir.ActivationFunctionType

# causal column stream: pass j emits scoresT columns for q in [128j, S)
OFFS = [0]
for j in range(NB):
    OFFS.append(OFFS[-1] + (S - 128 * j))
W = OFFS[NB]  # 17408 total causal columns per head


def build_program() -> bass.Bass:
    nc = bacc.Bacc(
        "TRN2", target_bir_lowering=False, debug=False, num_devices=NCORES
    )

    qt_d = nc.dram_tensor("qt", [HPC, D, S], F32R, kind="ExternalInput")
    kt_d = nc.dram_tensor("kt", [HPC, D, S], F32R, kind="ExternalInput")
    vt_d = nc.dram_tensor("vt", [HPC, D, S], F32, kind="ExternalInput")
    wqk_d = nc.dram_tensor("wqk", [HPC, D, D], F32R, kind="ExternalInput")
    wv_d = nc.dram_tensor("wv", [HPC, D, D], F32, kind="ExternalInput")
    identb_d = nc.dram_tensor("identb", [D, D], BF16, kind="ExternalInput")
    maskb_d = nc.dram_tensor("maskb", [D, D], BF16, kind="ExternalInput")
    out_d = nc.dram_tensor("out", [HPC, S, D], F32, kind="ExternalOutput")

    ntiles = (W + TILE - 1) // TILE
    tiles = [(t0, min(TILE, W - t0)) for t0 in range(0, W, TILE)]
    band_tile = [OFFS[i] // TILE for i in range(NB)]

    # input dma chunking (columns): small lead chunks so several dma
    # engines run them concurrently, then 512-col bulk chunks
    QK_CHUNKS = [(0, 128), (128, 128), (256, 256), (512, 512), (1024, 512),
                 (1536, 512)]
    VT_CHUNKS = [(0, 512), (512, 512), (1024, 512), (1536, 512)]

    with tile.TileContext(nc) as tc, ExitStack() as ctx:
        cpool = ctx.enter_context(tc.tile_pool(name="const", bufs=1))
        sb = ctx.enter_context(tc.tile_pool(name="sb", bufs=1))
        ps = ctx.enter_context(tc.tile_pool(name="ps", bufs=1, space="PSUM"))

        identb = cpool.tile([D, D], BF16)
        nc.sync.dma_start(identb[:], identb_d[:])
        maskb = cpool.tile([D, D], BF16)
        nc.sync.dma_start(maskb[:], maskb_d[:])

        # ---- all input DMAs up front, priority-ordered ----------------
        wqk_t, wv_t, qkraw, vtr = {}, {}, {}, {}
        for h in range(HPC):
            wqk_t[h] = sb.tile([D, D], F32R, tag="wqk", bufs=2, name=f"wqk_{h}")
            nc.sync.dma_start(wqk_t[h][:], wqk_d[h])
            wv_t[h] = sb.tile([D, D], F32, tag="wv", bufs=2, name=f"wv_{h}")
            nc.sync.dma_start(wv_t[h][:], wv_d[h])
            qkraw[h] = sb.tile([D, 2 * S], F32R, tag="qkraw", bufs=2,
                               name=f"qkraw_{h}")
            vtr[h] = sb.tile([D, S], F32, tag="vtr", bufs=2, name=f"vtr_{h}")
            # interleave q/k lead chunks, then v, then the bulk
            for c0, w in QK_CHUNKS[:2]:
                nc.sync.dma_start(qkraw[h][:, c0 : c0 + w], qt_d[h][:, c0 : c0 + w])
                nc.sync.dma_start(
                    qkraw[h][:, S + c0 : S + c0 + w], kt_d[h][:, c0 : c0 + w]
                )
            nc.sync.dma_start(vtr[h][:, 0:512], vt_d[h][:, 0:512])
            for c0, w in QK_CHUNKS[2:]:
                nc.sync.dma_start(qkraw[h][:, c0 : c0 + w], qt_d[h][:, c0 : c0 + w])
                nc.sync.dma_start(
                    qkraw[h][:, S + c0 : S + c0 + w], kt_d[h][:, c0 : c0 + w]
                )
            for c0, w in VT_CHUNKS[1:]:
                nc.sync.dma_start(vtr[h][:, c0 : c0 + w], vt_d[h][:, c0 : c0 + w])

        # ---- early gpsimd work: head-1 v-path casts + both memsets ----
        vtb, wvb, vaug, qkmt, exp_all, osb = {}, {}, {}, {}, {}, {}
        for h in range(HPC):
            vtb[h] = sb.tile([D, S], BF16, tag="vtb", bufs=2, name=f"vtb_{h}")
            wvb[h] = sb.tile([D, D], BF16, tag="wvb", bufs=2, name=f"wvb_{h}")
            vaug[h] = sb.tile([D, NB, 132], BF16, tag="vaug", bufs=2,
                              name=f"vaug_{h}")
            qkmt[h] = sb.tile([D, 2 * S], BF16, tag="qkmt", bufs=2,
                              name=f"qkmt_{h}")
            exp_all[h] = sb.tile([D, W], BF16, tag="expall", bufs=2,
                                 name=f"expall_{h}")
            osb[h] = sb.tile([D, NB, 128], F32, tag="osb", bufs=2,
                             name=f"osb_{h}")
            nc.gpsimd.memset(vaug[h][:, :, 128:129], 1.0)
        # head-1 vtb/wvb on gpsimd (idle early; needed only mid-kernel)
        nc.gpsimd.tensor_copy(wvb[1][:], wv_t[1][:])
        for c0, w in VT_CHUNKS:
            nc.gpsimd.tensor_copy(vtb[1][:, c0 : c0 + w], vtr[1][:, c0 : c0 + w])
        # head-0 vtb/wvb on DVE (gpsimd too slow to make tile-1 deadline)
        nc.vector.tensor_copy(wvb[0][:], wv_t[0][:])

        def emit_qk_proj(h, dve_all):
            """fp32r projections, 512-col chunks; psum drains split
            DVE(q)/ACT(k) for h0 (ACT idle pre-stream), all-DVE for h1."""
            for part in range(2):  # 0 = q cols, 1 = k cols
                base = part * S
                for c0 in range(0, S, 512):
                    bigt = ps.tile([D, TILE], F32, tag="big", bufs=2,
                                   name=f"projps_{h}_{part}_{c0}")
                    nc.tensor.matmul(
                        bigt[:, 0:512],
                        wqk_t[h][:],
                        qkraw[h][:, base + c0 : base + c0 + 512],
                        start=True,
                        stop=True,
                    )
                    if part == 0 and h == 0:
                        # interleave h0 vtb cast chunks on DVE
                        nc.vector.tensor_copy(
                            vtb[0][:, c0 : c0 + 512], vtr[0][:, c0 : c0 + 512]
                        )
                    if part == 1 and not dve_all:
                        nc.scalar.activation(
                            qkmt[h][:, base + c0 : base + c0 + 512],
                            bigt[:, 0:512], AF.Copy,
                        )
                    else:
                        nc.vector.tensor_copy(
                            qkmt[h][:, base + c0 : base + c0 + 512],
                            bigt[:, 0:512],
                        )

        def emit_vproj(h, j0, nj):
            """v chunks j0..j0+nj-1 -> vaug natural layout."""
            w = nj * 128
            bigt = ps.tile([D, TILE], F32, tag="big", bufs=2,
                           name=f"vps_{h}_{j0}")
            for c in range(0, w, 128):
                nc.tensor.matmul(
                    bigt[:, c : c + 128],
                    vtb[h][:, j0 * 128 + c : j0 * 128 + c + 128],
                    wvb[h][:],
                    start=True,
                    stop=True,
                )
            nc.vector.tensor_copy(
                vaug[h][:, j0 : j0 + nj, 0:128],
                bigt[:, 0:w].rearrange("p (j c) -> p j c", c=128),
            )

        def emit_pv(h, i, last_head):
            ctxp = ps.tile([D, 132], F32, tag="ctx", bufs=2,
                           name=f"ctx_{h}_{i}")
            for j in range(i + 1):
                p = OFFS[j] + 128 * (i - j)
                nc.tensor.matmul(
                    ctxp[:, 0:129],
                    exp_all[h][:, p : p + 128],
                    vaug[h][:, j, 0:129],
                    start=(j == 0),
                    stop=(j == i),
                )
            recip = sb.tile([D, 1], F32, tag="recip", bufs=2,
                            name=f"recip_{h}_{i}")
            nc.vector.reciprocal(recip[:], ctxp[:, 128:129])
            nc.vector.tensor_scalar_mul(
                osb[h][:, i, :], ctxp[:, 0:128], recip[:]
            )
            # milestone out-DMAs; split the tail bands on the last head so
            # the final DMA is small
            def dma_bands(b0, b1):
                nc.sync.dma_start(
                    out_d[h, b0 * 128 : b1 * 128, :].rearrange(
                        "(b s) e -> s b e", b=b1 - b0
                    ),
                    osb[h][:, b0:b1, :],
                )
            if i in (3, 7, 11):
                dma_bands(i - 3, i + 1)
            elif not last_head and i == 15:
                dma_bands(12, 16)
            elif last_head:
                if i == 13:
                    dma_bands(12, 14)
                elif i == 14:
                    dma_bands(14, 15)
                elif i == 15:
                    nc.sync.dma_start(out_d[h, 15 * 128 : 15 * 128 + 128, 0:64],
                                      osb[h][:, 15, 0:64])
                    nc.sync.dma_start(out_d[h, 15 * 128 : 15 * 128 + 128, 64:128],
                                      osb[h][:, 15, 64:128])

        def emit_score_tile(h, t):
            t0, w = tiles[t]
            bigt = ps.tile([D, TILE], F32, tag="big", bufs=2,
                           name=f"scps_{h}_{t}")
            pos = t0
            while pos < t0 + w:
                j = 0
                while OFFS[j + 1] <= pos:
                    j += 1
                bank_end = t0 + ((pos - t0) // 512 + 1) * 512
                end = min(OFFS[j + 1], t0 + w, bank_end)
                qcol = 128 * j + (pos - OFFS[j])
                is_start = pos == OFFS[j]
                nc.tensor.matmul(
                    bigt[:, pos - t0 : end - t0],
                    qkmt[h][:, S + 128 * j : S + 128 * (j + 1)],
                    qkmt[h][:, qcol : qcol + (end - pos)],
                    start=True,
                    stop=not is_start,
                )
                if is_start:
                    nc.tensor.matmul(
                        bigt[:, pos - t0 : pos - t0 + 128],
                        identb[:],
                        maskb[:],
                        start=False,
                        stop=True,
                    )
                pos = end
            nc.scalar.activation(
                exp_all[h][:, t0 : t0 + w], bigt[:, 0:w], AF.Exp,
                scale=1.0 / NF,
            )

        def emit_stream(h, interleave):
            """scores/exp/PV stream for head h.  `interleave` maps tile
            index -> list of callables to emit after that tile's exp."""
            issued = 0  # bands 0..issued-1 already emitted
            for t in range(ntiles):
                emit_score_tile(h, t)
                for fn in interleave.get(t, ()):
                    fn()
                if t >= 2:
                    while issued < NB and band_tile[issued] <= t - 1:
                        emit_pv(h, issued, h == HPC - 1)
                        issued += 1
            while issued < NB:
                emit_pv(h, issued, h == HPC - 1)
                issued += 1

        # ---- head 0: proj pre-stream, h1-proj interleaved late --------
        emit_qk_proj(0, dve_all=False)
        h1_proj_parts = []
        for part in range(2):
            for half in range(2):
                h1_proj_parts.append((part, half))

        def h1_proj_piece(part, half):
            def fn():
                base = part * S
                bigt = ps.tile([D, TILE], F32, tag="big", bufs=2,
                               name=f"projps1_{part}_{half}")
                for c in range(2):
                    c0 = half * 1024 + c * 512
                    nc.tensor.matmul(
                        bigt[:, c * 512 : c * 512 + 512],
                        wqk_t[1][:],
                        qkraw[1][:, base + c0 : base + c0 + 512],
                        start=True,
                        stop=True,
                    )
                nc.vector.tensor_copy(
                    qkmt[1][:, base + half * 1024 : base + half * 1024 + 1024],
                    bigt[:, 0:1024],
                )
            return fn

        inter0 = {
            1: [lambda: emit_vproj(0, 0, 12)],
            2: [lambda: emit_vproj(0, 12, 4)],
            7: [h1_proj_piece(0, 0)],
            8: [h1_proj_piece(0, 1)],
            9: [h1_proj_piece(1, 0)],
            10: [h1_proj_piece(1, 1)],
        }
        emit_stream(0, inter0)

        # ---- head 1 ---------------------------------------------------
        inter1 = {
            1: [lambda: emit_vproj(1, 0, 12)],
            2: [lambda: emit_vproj(1, 12, 4)],
        }
        emit_stream(1, inter1)

    nc.compile()
    return nc


_NC_CACHE = None


def _get_program():
    global _NC_CACHE
    if _NC_CACHE is None:
        _NC_CACHE = build_program()
    return _NC_CACHE


def make_in_maps(query_layer, key_layer, value_layer, svd_qk, svd_v):
    qt = np.ascontiguousarray(query_layer[:, 0].transpose(1, 2, 0))
    kt = np.ascontiguousarray(key_layer[:, 0].transpose(1, 2, 0))
    vt = np.ascontiguousarray(value_layer[:, 0].transpose(1, 2, 0))
    svd_qk = np.ascontiguousarray(svd_qk, dtype=np.float32)
    svd_v = np.ascontiguousarray(svd_v, dtype=np.float32)

    identb = np.eye(D, dtype=ml_dtypes.bfloat16)
    r = np.arange(D)
    maskb = np.where(r[:, None] > r[None, :], NEG, 0.0).astype(ml_dtypes.bfloat16)

    in_maps = []
    for c in range(NCORES):
        hs = slice(c * HPC, (c + 1) * HPC)
        in_maps.append(
            {
                "qt": qt[hs],
                "kt": kt[hs],
                "vt": vt[hs],
                "wqk": svd_qk[hs],
                "wv": svd_v[hs],
                "identb": identb,
                "maskb": maskb,
            }
        )
    return in_maps


def assemble_output(results):
    out = np.empty((S, B, H * D), dtype=np.float32)
    for c in range(NCORES):
        o = results[c]["out"]  # [HPC, S, D]
        for hl in range(HPC):
            h = c * HPC + hl
            out[:, 0, h * D : (h + 1) * D] = o[hl]
    return out


def kernel(query_layer, key_layer, value_layer, attention_mask, svd_qk, svd_v):
    nc = _get_program()
    in_maps = make_in_maps(query_layer, key_layer, value_layer, svd_qk, svd_v)
    res = run_bass_kernel_spmd(nc, in_maps, list(range(NCORES))).results
    return assemble_output(res)
